# revision 3
# baseline (speedup 1.0000x reference)
"""Trainium2 Bass kernel for nn_BranchingLayer (gnn_message_passing).

Computation (reference):
    parents_ftxs = x[idxs_level]                      # identity gather (arange)
    pg           = global_features[parents_idxs % B]  # random gather
    h1 = leaky_relu([parents_ftxs, pg] @ W1 + b1)
    h2 = h1 @ W2 + b2 + repeat(parents_ftxs, 2, -1)
    children = interleave-reshape(h2)                 # child (2p+br)*B+b, f <- h2[p*B+b, br*F+f]
    out = concat([x, children])

Device strategy (8 cores, 32768 rows/core), fully transposed dataflow:
  - host: per core, x^T -> [128, RPC] bf16 and gathered global^T -> [64, RPC]
    bf16 (natural column order, column j == local row j).
  - per 512-row group: mm1 (4 MMs, N=512) -> h1^T in PSUM; leaky-relu+b1 on
    ACT -> bf16 SBUF; mm2 with W2 chunk stationaries -> h2^T = children^T per
    branch in PSUM (4 MMs, N=512); DVE casts PSUM->SBUF bf16; DMA out to
    ch{br} [128, RPC].  PE issue order is mm1(g) then mm2(g-1) so the
    activation of group g overlaps mm2(g-1) and the PE never stalls.
  - host: un-transpose children, add exact f32 residual repeat(x,2) + b2,
    concat [x, children].
"""

import sys

import numpy as np

try:
    import ml_dtypes
except ImportError:
    ml_dtypes = None

if "/opt/trn_rl_repo" not in sys.path:
    sys.path.insert(0, "/opt/trn_rl_repo")

N_PARENTS = 256
BATCH = 1024
N_FEAT = 128
N_BR = 2
N_GLOBAL = 64
N_CORES = 8
ROWS = N_PARENTS * BATCH            # 262144
RPC = ROWS // N_CORES               # 32768 rows per core
PPC = N_PARENTS // N_CORES          # 32 parents per core
CPC = RPC * N_BR                    # 65536 child rows per core
GROUP = 512                         # rows per pipeline group
N_GROUPS = RPC // GROUP             # 64
HID = 256

_CACHE = {}


def _split_multiwait(nc, mybir):
    """This image's walrus accepts only one sync-wait per instruction; hoist
    extra waits onto same-engine NOPs inserted before the instruction."""
    for f in nc.m.functions:
        for bb in f.blocks:
            new_insts = []
            changed = False
            for inst in bb.instructions:
                si = inst.sync_info
                if si is not None and len(si.on_wait) > 1:
                    waits = list(si.on_wait)
                    for w in waits[:-1]:
                        new_insts.append(
                            mybir.InstNoOp(
                                name=nc.get_next_instruction_name(),
                                engine=inst.engine,
                                sync_info=mybir.SyncInfo(on_wait=[w], on_update=[]),
                            )
                        )
                    inst.sync_info = mybir.SyncInfo(
                        on_wait=[waits[-1]], on_update=list(si.on_update)
                    )
                    changed = True
                new_insts.append(inst)
            if changed:
                bb.instructions = new_insts


def _build_program(split_waits=True):
    key = ("prog_v2", split_waits)
    if key in _CACHE:
        return _CACHE[key]

    import concourse.bass as bass
    import concourse.mybir as mybir
    import concourse.tile as tile

    f32 = mybir.dt.float32
    bf16 = mybir.dt.bfloat16
    AF = mybir.ActivationFunctionType

    nc = bass.Bass()
    xt = nc.declare_dram_parameter("xt", [N_FEAT, RPC], bf16, isOutput=False)
    pgt = nc.declare_dram_parameter("pgt", [N_GLOBAL, RPC], bf16, isOutput=False)
    w1x = nc.declare_dram_parameter("w1x", [N_FEAT, HID], bf16, isOutput=False)
    w1g = nc.declare_dram_parameter("w1g", [N_GLOBAL, HID], bf16, isOutput=False)
    w2p = nc.declare_dram_parameter("w2p", [HID, HID], bf16, isOutput=False)
    b1c = nc.declare_dram_parameter("b1c", [128, 2], f32, isOutput=False)
    chd = [
        nc.declare_dram_parameter(f"ch{br}", [N_FEAT, RPC], bf16, isOutput=True)
        for br in range(N_BR)
    ]

    with tile.TileContext(nc) as tc:
        with (
            tc.tile_pool(name="const", bufs=1) as cpool,
            tc.tile_pool(name="xin", bufs=4) as xpool,
            tc.tile_pool(name="pg", bufs=4) as gpool,
            tc.tile_pool(name="h1", bufs=3) as h1pool,
            tc.tile_pool(name="cout", bufs=4) as opool,
            tc.tile_pool(name="psB", bufs=4, space="PSUM") as psB,
            tc.tile_pool(name="psC", bufs=4, space="PSUM") as psC,
        ):
            w1xs = cpool.tile([N_FEAT, HID], bf16)
            nc.sync.dma_start(w1xs[:], w1x[:, :])
            w1gs = cpool.tile([N_GLOBAL, HID], bf16)
            nc.sync.dma_start(w1gs[:], w1g[:, :])
            w2s = [cpool.tile([128, HID], bf16, name=f"w2s{k}") for k in range(2)]
            for k in range(2):
                nc.sync.dma_start(w2s[k][:], w2p[k * 128:(k + 1) * 128, :])
            b1s = cpool.tile([128, 2], f32)
            nc.sync.dma_start(b1s[:], b1c[:])

            prev = None
            for g in range(N_GROUPS + 1):
                cur = None
                if g < N_GROUPS:
                    sl = slice(g * GROUP, (g + 1) * GROUP)
                    xtg = xpool.tile([N_FEAT, GROUP], bf16)
                    nc.sync.dma_start(xtg[:, :], xt[:, sl])
                    pgg = gpool.tile([N_GLOBAL, GROUP], bf16)
                    nc.sync.dma_start(pgg[:, :], pgt[:, sl])

                    # ---- mm1(g): h1^T [hid, rows] in PSUM ----
                    h1ps = [psB.tile([128, GROUP], f32, tag="h1ps", name=f"h1ps{m_}")
                            for m_ in range(2)]
                    cur = {"g": g, "h1ps": h1ps}
                    for m in range(2):
                        nc.tensor.matmul(
                            h1ps[m][:, :], w1xs[:, m * 128:(m + 1) * 128], xtg[:, :],
                            start=True, stop=False,
                        )
                    for m in range(2):
                        nc.tensor.matmul(
                            h1ps[m][:, :], w1gs[:, m * 128:(m + 1) * 128], pgg[:, :],
                            start=False, stop=True,
                        )

                h2ps = None
                if prev is not None:
                    # ---- mm2(g-1): children^T per branch in PSUM ----
                    h2ps = [psC.tile([128, GROUP], f32, tag="h2ps", name=f"h2ps{br_}")
                            for br_ in range(2)]
                    h1sb = prev["h1sb"]
                    for br in range(2):
                        for k in range(2):
                            nc.tensor.matmul(
                                h2ps[br][:, :],
                                w2s[k][:, br * 128:(br + 1) * 128],
                                h1sb[k][:, :],
                                start=(k == 0), stop=(k == 1),
                            )

                if cur is not None:
                    # ---- ACT: leaky-relu(g) -> bf16, overlaps mm2(g-1) ----
                    h1sb = [h1pool.tile([128, GROUP], bf16, tag="h1sb",
                                        name=f"h1sb{m_}") for m_ in range(2)]
                    for m in range(2):
                        nc.scalar.activation(
                            h1sb[m][:, :], cur["h1ps"][m][:, :], AF.Lrelu,
                            bias=b1s[:, m:m + 1], scale=1.0, alpha=0.01,
                        )
                    cur["h1sb"] = h1sb

                if prev is not None:
                    # ---- DVE cast + store children^T (g-1) ----
                    slp = slice(prev["g"] * GROUP, (prev["g"] + 1) * GROUP)
                    for br in range(2):
                        cho = opool.tile([128, GROUP], bf16, tag="cho",
                                         name=f"cho{br}")
                        nc.vector.tensor_copy(cho[:, :], h2ps[br][:, :])
                        nc.sync.dma_start(chd[br][:, slp], cho[:, :])

                prev = cur

    if split_waits:
        _split_multiwait(nc, mybir)
    _CACHE[key] = nc
    return nc


def _host_prep(x, global_features, W1, b1, W2, b2, idxs_level, parents_idxs):
    bf = ml_dtypes.bfloat16
    x = np.ascontiguousarray(np.asarray(x, dtype=np.float32))
    G = np.asarray(global_features, dtype=np.float32)
    W1 = np.asarray(W1, dtype=np.float32)
    b1 = np.asarray(b1, dtype=np.float32)
    W2 = np.asarray(W2, dtype=np.float32)
    idxs = np.asarray(idxs_level)
    pidx = np.asarray(parents_idxs)

    if np.array_equal(idxs, np.arange(ROWS, dtype=idxs.dtype)):
        xg = x
    else:  # general gather fallback (host)
        xg = np.ascontiguousarray(x[idxs])

    # transposed per-core x: [8, 128, RPC] bf16, column j == local row j
    xtv = np.ascontiguousarray(
        xg.reshape(N_CORES, RPC, N_FEAT).transpose(0, 2, 1)
    ).astype(bf)
    # transposed per-core gathered globals: [8, 64, RPC] bf16
    pg = G[pidx % BATCH]                              # [ROWS, 64]
    pgtv = np.ascontiguousarray(
        pg.reshape(N_CORES, RPC, N_GLOBAL).transpose(0, 2, 1)
    ).astype(bf)

    w1xh = W1[:N_FEAT, :].astype(bf)
    w1gh = W1[N_FEAT:, :].astype(bf)
    w2h = W2.astype(bf)
    b1c = np.ascontiguousarray(b1.reshape(2, 128).T)  # [128, 2]

    in_maps = []
    for c in range(N_CORES):
        in_maps.append({
            "xt": xtv[c],
            "pgt": pgtv[c],
            "w1x": w1xh,
            "w1g": w1gh,
            "w2p": w2h,
            "b1c": b1c,
        })
    return xg, in_maps


def _host_post(xg, results, b2):
    """Assemble full output: [x ; children], adding the exact f32 residual
    repeat(x, 2, axis=-1) and b2 on host."""
    b2 = np.asarray(b2, dtype=np.float32)
    out = np.empty((ROWS + ROWS * N_BR, N_FEAT), dtype=np.float32)
    out[:ROWS] = xg
    # child (global): core c, local parent p, branch br, batch b:
    #   row ROWS + c*CPC + (2p+br)*B + b
    chv = out[ROWS:].reshape(N_CORES, PPC, N_BR, BATCH, N_FEAT)
    xr = xg.reshape(N_CORES, PPC, BATCH, N_FEAT)
    rep_idx = np.arange(N_FEAT) // 2      # residual: channel br*128+f <- x[64*br + f//2]
    for c in range(N_CORES):
        for br in range(N_BR):
            ffn = results[c][f"ch{br}"].astype(np.float32)   # [128, RPC]
            ffn = ffn.T.reshape(PPC, BATCH, N_FEAT)
            res = xr[c][:, :, 64 * br + rep_idx]
            chv[c, :, br] = ffn + res + b2[br * N_FEAT:(br + 1) * N_FEAT]
    return out


def kernel(x, global_features, W1, b1, W2, b2, idxs_level, parents_idxs,
           _trace=False, _trace_kwargs=None):
    from concourse.bass_utils import run_bass_kernel_spmd

    xg, in_maps = _host_prep(
        x, global_features, W1, b1, W2, b2, idxs_level, parents_idxs
    )
    nc = _build_program()
    res = run_bass_kernel_spmd(
        nc, in_maps, list(range(N_CORES)),
        trace=_trace, **(_trace_kwargs or {}),
    )
    out = _host_post(xg, res.results, b2)
    if _trace:
        kernel.last_result = res
    return out


# revision 4
# speedup vs baseline: 1.2798x; 1.2798x over previous
"""Trainium2 Bass kernel for nn_BranchingLayer (gnn_message_passing).

Computation (reference):
    parents_ftxs = x[idxs_level]                      # identity gather (arange)
    pg           = global_features[parents_idxs % B]  # random gather
    h1 = leaky_relu([parents_ftxs, pg] @ W1 + b1)
    h2 = h1 @ W2 + b2 + repeat(parents_ftxs, 2, -1)
    children = interleave-reshape(h2)                 # child (2p+br)*B+b, f <- h2[p*B+b, br*F+f]
    out = concat([x, children])

Device strategy (8 cores, 32768 rows/core), fully transposed bf16 dataflow:
  - host: per core, x^T -> [128, RPC] bf16 and gathered global^T -> [64, RPC]
    bf16 (natural column order, column j == local row j).
  - compute in 2-group chunks (group = 512 rows): mm1 (8 MMs, N=512,
    stationary-major so consecutive MMs share lhsT) -> h1^T [128,1024] PSUM
    per group; fused leaky-relu on ACT -> bf16 SBUF; mm2 with W2 chunk
    stationaries -> children^T per branch [128,1024] PSUM per group; fused
    DVE cast -> [128, 2048] bf16 SBUF per chunk; single out-DMA per chunk.
    Input DMAs at 4-group granularity, prefetched one chunk-pair ahead.
    PE issue order is mm1(c) then mm2(c-1) so ACT/DVE overlap the PE.
  - host: un-transpose children, add exact f32 residual repeat(x,2) + b2,
    concat [x, children].
"""

import sys

import numpy as np

try:
    import ml_dtypes
except ImportError:
    ml_dtypes = None

if "/opt/trn_rl_repo" not in sys.path:
    sys.path.insert(0, "/opt/trn_rl_repo")

N_PARENTS = 256
BATCH = 1024
N_FEAT = 128
N_BR = 2
N_GLOBAL = 64
N_CORES = 8
ROWS = N_PARENTS * BATCH            # 262144
RPC = ROWS // N_CORES               # 32768 rows per core
PPC = N_PARENTS // N_CORES          # 32 parents per core
CPC = RPC * N_BR                    # 65536 child rows per core
GROUP = 512                         # rows per compute group
N_GROUPS = RPC // GROUP             # 64
N_CHUNKS = N_GROUPS // 2            # 32 compute chunks (2 groups each)
N_PAIRS = N_CHUNKS // 2             # 16 input-DMA pairs (4 groups each)
HID = 256

_CACHE = {}


def _split_multiwait(nc, mybir):
    """This image's walrus accepts only one sync-wait per instruction; hoist
    extra waits onto same-engine NOPs inserted before the instruction."""
    for f in nc.m.functions:
        for bb in f.blocks:
            new_insts = []
            changed = False
            for inst in bb.instructions:
                si = inst.sync_info
                if si is not None and len(si.on_wait) > 1:
                    waits = list(si.on_wait)
                    for w in waits[:-1]:
                        new_insts.append(
                            mybir.InstNoOp(
                                name=nc.get_next_instruction_name(),
                                engine=inst.engine,
                                sync_info=mybir.SyncInfo(on_wait=[w], on_update=[]),
                            )
                        )
                    inst.sync_info = mybir.SyncInfo(
                        on_wait=[waits[-1]], on_update=list(si.on_update)
                    )
                    changed = True
                new_insts.append(inst)
            if changed:
                bb.instructions = new_insts


def _build_program(b1_zero=True, split_waits=True):
    key = ("prog_v3", b1_zero, split_waits)
    if key in _CACHE:
        return _CACHE[key]

    import concourse.bass as bass
    import concourse.mybir as mybir
    import concourse.tile as tile

    f32 = mybir.dt.float32
    bf16 = mybir.dt.bfloat16
    AF = mybir.ActivationFunctionType

    nc = bass.Bass()
    xt = nc.declare_dram_parameter("xt", [N_FEAT, RPC], bf16, isOutput=False)
    pgt = nc.declare_dram_parameter("pgt", [N_GLOBAL, RPC], bf16, isOutput=False)
    w1x = nc.declare_dram_parameter("w1x", [N_FEAT, HID], bf16, isOutput=False)
    w1g = nc.declare_dram_parameter("w1g", [N_GLOBAL, HID], bf16, isOutput=False)
    w2p = nc.declare_dram_parameter("w2p", [HID, HID], bf16, isOutput=False)
    b1c = nc.declare_dram_parameter("b1c", [128, 2], f32, isOutput=False)
    ch = nc.declare_dram_parameter("ch", [N_FEAT, N_BR * RPC], bf16, isOutput=True)
    ch_v = ch.rearrange("p (b r) -> p b r", b=N_BR)

    with tile.TileContext(nc) as tc:
        with (
            tc.tile_pool(name="const", bufs=1) as cpool,
            tc.tile_pool(name="xin", bufs=2) as xpool,
            tc.tile_pool(name="pg", bufs=2) as gpool,
            tc.tile_pool(name="h1", bufs=3) as h1pool,
            tc.tile_pool(name="cout", bufs=3) as opool,
            tc.tile_pool(name="psB", bufs=2, space="PSUM") as psB,
            tc.tile_pool(name="psC", bufs=2, space="PSUM") as psC,
        ):
            w1xs = cpool.tile([N_FEAT, HID], bf16)
            nc.sync.dma_start(w1xs[:], w1x[:, :])
            w1gs = cpool.tile([N_GLOBAL, HID], bf16)
            nc.sync.dma_start(w1gs[:], w1g[:, :])
            w2s = [cpool.tile([128, HID], bf16, name=f"w2s{k}") for k in range(2)]
            for k in range(2):
                nc.sync.dma_start(w2s[k][:], w2p[k * 128:(k + 1) * 128, :])
            b1s = cpool.tile([128, 2], f32)
            nc.sync.dma_start(b1s[:], b1c[:])

            PW = 4 * GROUP          # columns per input pair (4 groups)
            CW = 2 * GROUP          # columns per compute chunk (2 groups)

            def issue_in(p):
                xtc = xpool.tile([N_FEAT, PW], bf16, tag="xtc", name=f"xtc{p % 2}")
                nc.sync.dma_start(xtc[:, :], xt[:, p * PW:(p + 1) * PW])
                pgc = gpool.tile([N_GLOBAL, PW], bf16, tag="pgc", name=f"pgc{p % 2}")
                nc.sync.dma_start(pgc[:, :], pgt[:, p * PW:(p + 1) * PW])
                return xtc, pgc

            pair_tiles = {0: issue_in(0)}

            prev = None
            for c in range(N_CHUNKS + 1):
                cur = None
                if c < N_CHUNKS:
                    if c % 2 == 0 and (c // 2 + 1) < N_PAIRS:
                        pair_tiles[c // 2 + 1] = issue_in(c // 2 + 1)
                    xtc, pgc = pair_tiles[c // 2]
                    off = (c % 2) * CW
                    xg = [xtc[:, off + g * GROUP: off + (g + 1) * GROUP]
                          for g in range(2)]
                    pgg = [pgc[:, off + g * GROUP: off + (g + 1) * GROUP]
                           for g in range(2)]

                    # ---- mm1(c): h1^T per group, stationary-major ----
                    h1ps = [psB.tile([128, CW], f32, tag="h1ps", name=f"h1ps{g_}")
                            for g_ in range(2)]
                    cur = {"c": c, "h1ps": h1ps}
                    for m in range(2):
                        for g in range(2):
                            nc.tensor.matmul(
                                h1ps[g][:, m * GROUP:(m + 1) * GROUP],
                                w1xs[:, m * 128:(m + 1) * 128], xg[g],
                                start=True, stop=False,
                            )
                    for m in range(2):
                        for g in range(2):
                            nc.tensor.matmul(
                                h1ps[g][:, m * GROUP:(m + 1) * GROUP],
                                w1gs[:, m * 128:(m + 1) * 128], pgg[g],
                                start=False, stop=True,
                            )

                h2ps = None
                if prev is not None:
                    # ---- mm2(c-1): children^T per branch, stationary-major ----
                    h2ps = [psC.tile([128, CW], f32, tag="h2ps", name=f"h2ps{g_}")
                            for g_ in range(2)]
                    h1sb = prev["h1sb"]
                    for br in range(2):
                        for k in range(2):
                            for g in range(2):
                                nc.tensor.matmul(
                                    h2ps[g][:, br * GROUP:(br + 1) * GROUP],
                                    w2s[k][:, br * 128:(br + 1) * 128],
                                    h1sb[g][:, k * GROUP:(k + 1) * GROUP],
                                    start=(k == 0), stop=(k == 1),
                                )

                if cur is not None:
                    # ---- ACT: fused leaky-relu(c) -> bf16, overlaps mm2(c-1) ----
                    h1sb = [h1pool.tile([128, CW], bf16, tag="h1sb",
                                        name=f"h1sb{g_}") for g_ in range(2)]
                    for g in range(2):
                        if b1_zero:
                            nc.scalar.activation(
                                h1sb[g][:, :], cur["h1ps"][g][:, :], AF.Lrelu,
                                bias=0.0, scale=1.0, alpha=0.01,
                            )
                        else:
                            for m in range(2):
                                ms = slice(m * GROUP, (m + 1) * GROUP)
                                nc.scalar.activation(
                                    h1sb[g][:, ms], cur["h1ps"][g][:, ms], AF.Lrelu,
                                    bias=b1s[:, m:m + 1], scale=1.0, alpha=0.01,
                                )
                    cur["h1sb"] = h1sb

                if prev is not None:
                    # ---- DVE cast (c-1) + single out-DMA per chunk ----
                    pc = prev["c"]
                    cho = opool.tile([128, N_BR * CW], bf16, tag="cho")
                    cho4 = cho[:, :].rearrange("p (b g j) -> p g b j", b=2, g=2)
                    for g in range(2):
                        nc.vector.tensor_copy(
                            cho4[:, g, :, :],
                            h2ps[g][:, :].rearrange("p (b j) -> p b j", b=2),
                        )
                    nc.sync.dma_start(
                        ch_v[:, :, pc * CW:(pc + 1) * CW],
                        cho[:, :].rearrange("p (b t) -> p b t", b=2),
                    )

                prev = cur

    if split_waits:
        _split_multiwait(nc, mybir)
    _CACHE[key] = nc
    return nc


def _host_prep(x, global_features, W1, b1, W2, b2, idxs_level, parents_idxs):
    bf = ml_dtypes.bfloat16
    x = np.ascontiguousarray(np.asarray(x, dtype=np.float32))
    G = np.asarray(global_features, dtype=np.float32)
    W1 = np.asarray(W1, dtype=np.float32)
    b1 = np.asarray(b1, dtype=np.float32)
    W2 = np.asarray(W2, dtype=np.float32)
    idxs = np.asarray(idxs_level)
    pidx = np.asarray(parents_idxs)

    if np.array_equal(idxs, np.arange(ROWS, dtype=idxs.dtype)):
        xg = x
    else:  # general gather fallback (host)
        xg = np.ascontiguousarray(x[idxs])

    # transposed per-core x: [8, 128, RPC] bf16, column j == local row j
    xtv = np.ascontiguousarray(
        xg.reshape(N_CORES, RPC, N_FEAT).transpose(0, 2, 1)
    ).astype(bf)
    # transposed per-core gathered globals: [8, 64, RPC] bf16
    pg = G[pidx % BATCH]                              # [ROWS, 64]
    pgtv = np.ascontiguousarray(
        pg.reshape(N_CORES, RPC, N_GLOBAL).transpose(0, 2, 1)
    ).astype(bf)

    w1xh = W1[:N_FEAT, :].astype(bf)
    w1gh = W1[N_FEAT:, :].astype(bf)
    w2h = W2.astype(bf)
    b1c = np.ascontiguousarray(b1.reshape(2, 128).T)  # [128, 2]
    b1_zero = not np.any(b1)

    in_maps = []
    for c in range(N_CORES):
        in_maps.append({
            "xt": xtv[c],
            "pgt": pgtv[c],
            "w1x": w1xh,
            "w1g": w1gh,
            "w2p": w2h,
            "b1c": b1c,
        })
    return xg, in_maps, b1_zero


def _host_post(xg, results, b2):
    """Assemble full output: [x ; children], adding the exact f32 residual
    repeat(x, 2, axis=-1) and b2 on host."""
    b2 = np.asarray(b2, dtype=np.float32)
    out = np.empty((ROWS + ROWS * N_BR, N_FEAT), dtype=np.float32)
    out[:ROWS] = xg
    # child (global): core c, local parent p, branch br, batch b:
    #   row ROWS + c*CPC + (2p+br)*B + b
    chv = out[ROWS:].reshape(N_CORES, PPC, N_BR, BATCH, N_FEAT)
    xr = xg.reshape(N_CORES, PPC, BATCH, N_FEAT)
    rep_idx = np.arange(N_FEAT) // 2      # residual: channel br*128+f <- x[64*br + f//2]
    for c in range(N_CORES):
        chc = results[c]["ch"].reshape(N_FEAT, N_BR, RPC)
        for br in range(N_BR):
            ffn = chc[:, br, :].astype(np.float32)           # [128, RPC]
            ffn = np.ascontiguousarray(ffn.T).reshape(PPC, BATCH, N_FEAT)
            res = xr[c][:, :, 64 * br + rep_idx]
            chv[c, :, br] = ffn + res + b2[br * N_FEAT:(br + 1) * N_FEAT]
    return out


def kernel(x, global_features, W1, b1, W2, b2, idxs_level, parents_idxs,
           _trace=False, _trace_kwargs=None):
    from concourse.bass_utils import run_bass_kernel_spmd

    xg, in_maps, b1_zero = _host_prep(
        x, global_features, W1, b1, W2, b2, idxs_level, parents_idxs
    )
    nc = _build_program(b1_zero=b1_zero)
    res = run_bass_kernel_spmd(
        nc, in_maps, list(range(N_CORES)),
        trace=_trace, **(_trace_kwargs or {}),
    )
    out = _host_post(xg, res.results, b2)
    if _trace:
        kernel.last_result = res
    return out


# revision 12
# speedup vs baseline: 1.3064x; 1.0208x over previous
"""Trainium2 Bass kernel for nn_BranchingLayer (gnn_message_passing).

Computation (reference):
    parents_ftxs = x[idxs_level]                      # identity gather (arange)
    pg           = global_features[parents_idxs % B]  # random gather
    h1 = leaky_relu([parents_ftxs, pg] @ W1 + b1)
    h2 = h1 @ W2 + b2 + repeat(parents_ftxs, 2, -1)
    children = interleave-reshape(h2)                 # child (2p+br)*B+b, f <- h2[p*B+b, br*F+f]
    out = concat([x, children])

Device strategy (8 cores, 32768 rows/core), fully transposed bf16 dataflow:
  - host: per core, x^T -> [128, RPC] bf16 and gathered global^T -> [64, RPC]
    bf16 (natural column order, column j == local row j).
  - compute in 2-group chunks (group = 512 rows): mm1 (8 MMs, N=512,
    stationary-major so consecutive MMs share lhsT) -> h1^T [128,1024] PSUM
    per group; fused leaky-relu on ACT -> bf16 SBUF; mm2 with W2 chunk
    stationaries -> children^T per branch [128,1024] PSUM per group; fused
    DVE cast -> [128, 2048] bf16 SBUF per chunk; single out-DMA per chunk.
    Input DMAs at 4-group granularity, prefetched one chunk-pair ahead.
    PE issue order is mm1(c) then mm2(c-1) so ACT/DVE overlap the PE.
  - host: un-transpose children, add exact f32 residual repeat(x,2) + b2,
    concat [x, children].
"""

import sys

import numpy as np

try:
    import ml_dtypes
except ImportError:
    ml_dtypes = None

if "/opt/trn_rl_repo" not in sys.path:
    sys.path.insert(0, "/opt/trn_rl_repo")

N_PARENTS = 256
BATCH = 1024
N_FEAT = 128
N_BR = 2
N_GLOBAL = 64
N_CORES = 8
ROWS = N_PARENTS * BATCH            # 262144
RPC = ROWS // N_CORES               # 32768 rows per core
PPC = N_PARENTS // N_CORES          # 32 parents per core
CPC = RPC * N_BR                    # 65536 child rows per core
GROUP = 512                         # rows per compute group
N_GROUPS = RPC // GROUP             # 64
N_CHUNKS = N_GROUPS // 2            # 32 compute chunks (2 groups each)
N_PAIRS = N_CHUNKS // 2             # 16 input-DMA pairs (4 groups each)
HID = 256

_CACHE = {}


def _split_multiwait(nc, mybir):
    """This image's walrus accepts only one sync-wait per instruction; hoist
    extra waits onto same-engine NOPs inserted before the instruction."""
    for f in nc.m.functions:
        for bb in f.blocks:
            new_insts = []
            changed = False
            for inst in bb.instructions:
                si = inst.sync_info
                if si is not None and len(si.on_wait) > 1:
                    waits = list(si.on_wait)
                    for w in waits[:-1]:
                        new_insts.append(
                            mybir.InstNoOp(
                                name=nc.get_next_instruction_name(),
                                engine=inst.engine,
                                sync_info=mybir.SyncInfo(on_wait=[w], on_update=[]),
                            )
                        )
                    inst.sync_info = mybir.SyncInfo(
                        on_wait=[waits[-1]], on_update=list(si.on_update)
                    )
                    changed = True
                new_insts.append(inst)
            if changed:
                bb.instructions = new_insts


def _mark_ldweights_reuse(nc, mybir):
    """Post-schedule pass: the tile framework emits an InstLdweights before
    every matmul.  When consecutive loads target the identical stationary AP
    (only matmuls/NOPs between), the reload is redundant — drop it (or turn
    it into a NoOp if it carries sync waits/updates)."""
    def wkey(inst):
        w = inst.ins[0]
        return (w.memref, w.offset, tuple(tuple(p) for p in w.ap), str(w.dtype),
                bool(inst.is_transpose), str(getattr(inst, "perf_mode", None)))

    removed = 0
    for f in nc.m.functions:
        for bb in f.blocks:
            new_insts = []
            changed = False
            last = None
            for inst in bb.instructions:
                if inst.engine != mybir.EngineType.PE:
                    new_insts.append(inst)
                    continue
                if isinstance(inst, mybir.InstLdweights):
                    k = wkey(inst)
                    if last is not None and k == last:
                        removed += 1
                        changed = True
                        si = inst.sync_info
                        if si is not None and (si.on_wait or si.on_update):
                            new_insts.append(
                                mybir.InstNoOp(
                                    name=nc.get_next_instruction_name(),
                                    engine=inst.engine,
                                    sync_info=si,
                                )
                            )
                        continue
                    last = k
                elif isinstance(inst, (mybir.InstMatmult, mybir.InstNoOp)):
                    pass
                else:
                    last = None
                new_insts.append(inst)
            if changed:
                bb.instructions = new_insts
    return removed


def _build_program(b1_zero=True, split_waits=True):
    key = ("prog_v4", b1_zero, split_waits)
    if key in _CACHE:
        return _CACHE[key]

    import concourse.bass as bass
    import concourse.mybir as mybir
    import concourse.tile as tile

    f32 = mybir.dt.float32
    bf16 = mybir.dt.bfloat16
    AF = mybir.ActivationFunctionType

    nc = bass.Bass()
    xt = nc.declare_dram_parameter("xt", [N_FEAT, RPC], bf16, isOutput=False)
    pgt = nc.declare_dram_parameter("pgt", [N_GLOBAL, RPC], bf16, isOutput=False)
    w1x = nc.declare_dram_parameter("w1x", [N_FEAT, HID], bf16, isOutput=False)
    w1g = nc.declare_dram_parameter("w1g", [N_GLOBAL, HID], bf16, isOutput=False)
    w2p = nc.declare_dram_parameter("w2p", [HID, HID], bf16, isOutput=False)
    b1c = nc.declare_dram_parameter("b1c", [128, 2], f32, isOutput=False)
    ch = nc.declare_dram_parameter("ch", [N_FEAT, N_BR * RPC], bf16, isOutput=True)
    ch_v = ch.rearrange("p (b r) -> p b r", b=N_BR)

    with tile.TileContext(nc) as tc:
        with (
            tc.tile_pool(name="const", bufs=1) as cpool,
            tc.tile_pool(name="xin", bufs=2) as xpool,
            tc.tile_pool(name="pg", bufs=2) as gpool,
            tc.tile_pool(name="h1", bufs=3) as h1pool,
            tc.tile_pool(name="cout", bufs=3) as opool,
            tc.tile_pool(name="psB", bufs=2, space="PSUM") as psB,
            tc.tile_pool(name="psC", bufs=2, space="PSUM") as psC,
        ):
            w1xs = cpool.tile([N_FEAT, HID], bf16)
            nc.sync.dma_start(w1xs[:], w1x[:, :])
            w1gs = cpool.tile([N_GLOBAL, HID], bf16)
            nc.sync.dma_start(w1gs[:], w1g[:, :])
            w2s = [cpool.tile([128, HID], bf16, name=f"w2s{k}") for k in range(2)]
            for k in range(2):
                nc.sync.dma_start(w2s[k][:], w2p[k * 128:(k + 1) * 128, :])
            b1s = cpool.tile([128, 2], f32)
            nc.sync.dma_start(b1s[:], b1c[:])

            PW = 4 * GROUP          # columns per input pair (4 groups)
            CW = 2 * GROUP          # columns per compute chunk (2 groups)

            def issue_in(p):
                xtc = xpool.tile([N_FEAT, PW], bf16, tag="xtc", name=f"xtc{p % 2}")
                nc.sync.dma_start(xtc[:, :], xt[:, p * PW:(p + 1) * PW])
                pgc = gpool.tile([N_GLOBAL, PW], bf16, tag="pgc", name=f"pgc{p % 2}")
                nc.sync.dma_start(pgc[:, :], pgt[:, p * PW:(p + 1) * PW])
                return xtc, pgc

            pair_tiles = {0: issue_in(0)}

            prev = None
            for c in range(N_CHUNKS + 1):
                cur = None
                if c < N_CHUNKS:
                    if c % 2 == 0 and (c // 2 + 1) < N_PAIRS:
                        pair_tiles[c // 2 + 1] = issue_in(c // 2 + 1)
                    xtc, pgc = pair_tiles[c // 2]
                    off = (c % 2) * CW
                    xg = [xtc[:, off + g * GROUP: off + (g + 1) * GROUP]
                          for g in range(2)]
                    pgg = [pgc[:, off + g * GROUP: off + (g + 1) * GROUP]
                           for g in range(2)]

                    # ---- mm1(c): h1ps[m] spans both groups (cols g*512+j) so
                    # the scheduler's early-close order pairs stationaries ----
                    h1ps = [psB.tile([128, CW], f32, tag="h1ps", name=f"h1ps{m_}")
                            for m_ in range(2)]
                    cur = {"c": c, "h1ps": h1ps}
                    for m in range(2):
                        for g in range(2):
                            nc.tensor.matmul(
                                h1ps[m][:, g * GROUP:(g + 1) * GROUP],
                                w1xs[:, m * 128:(m + 1) * 128], xg[g],
                                start=True, stop=False,
                            )
                        for g in range(2):
                            nc.tensor.matmul(
                                h1ps[m][:, g * GROUP:(g + 1) * GROUP],
                                w1gs[:, m * 128:(m + 1) * 128], pgg[g],
                                start=False, stop=True,
                            )

                h2ps = None
                if prev is not None:
                    # ---- mm2(c-1): h2ps[br] spans both groups (cols g*512+j) ----
                    h2ps = [psC.tile([128, CW], f32, tag="h2ps", name=f"h2ps{br_}")
                            for br_ in range(2)]
                    h1sb = prev["h1sb"]
                    for br in range(2):
                        for k in range(2):
                            for g in range(2):
                                nc.tensor.matmul(
                                    h2ps[br][:, g * GROUP:(g + 1) * GROUP],
                                    w2s[k][:, br * 128:(br + 1) * 128],
                                    h1sb[k][:, g * GROUP:(g + 1) * GROUP],
                                    start=(k == 0), stop=(k == 1),
                                )

                if cur is not None:
                    # ---- ACT: fused leaky-relu(c) -> bf16, overlaps mm2(c-1).
                    # h1ps[m] has a single hid-half on partitions, so the b1
                    # bias is per-partition and exact for any b1. ----
                    h1sb = [h1pool.tile([128, CW], bf16, tag="h1sb",
                                        name=f"h1sb{m_}") for m_ in range(2)]
                    for m in range(2):
                        nc.scalar.activation(
                            h1sb[m][:, :], cur["h1ps"][m][:, :], AF.Lrelu,
                            bias=b1s[:, m:m + 1], scale=1.0, alpha=0.01,
                        )
                    cur["h1sb"] = h1sb

                if prev is not None:
                    # ---- DVE cast (c-1), contiguous per branch + one out-DMA ----
                    pc = prev["c"]
                    cho = opool.tile([128, N_BR * CW], bf16, tag="cho")
                    for br in range(2):
                        nc.vector.tensor_copy(
                            cho[:, br * CW:(br + 1) * CW], h2ps[br][:, :],
                        )
                    nc.sync.dma_start(
                        ch_v[:, :, pc * CW:(pc + 1) * CW],
                        cho[:, :].rearrange("p (b t) -> p b t", b=2),
                    )

                prev = cur

    if split_waits:
        _split_multiwait(nc, mybir)
    _mark_ldweights_reuse(nc, mybir)
    _CACHE[key] = nc
    return nc


def _host_prep(x, global_features, W1, b1, W2, b2, idxs_level, parents_idxs):
    bf = ml_dtypes.bfloat16
    x = np.ascontiguousarray(np.asarray(x, dtype=np.float32))
    G = np.asarray(global_features, dtype=np.float32)
    W1 = np.asarray(W1, dtype=np.float32)
    b1 = np.asarray(b1, dtype=np.float32)
    W2 = np.asarray(W2, dtype=np.float32)
    idxs = np.asarray(idxs_level)
    pidx = np.asarray(parents_idxs)

    if np.array_equal(idxs, np.arange(ROWS, dtype=idxs.dtype)):
        xg = x
    else:  # general gather fallback (host)
        xg = np.ascontiguousarray(x[idxs])

    # transposed per-core x: [8, 128, RPC] bf16, column j == local row j
    xtv = np.ascontiguousarray(
        xg.reshape(N_CORES, RPC, N_FEAT).transpose(0, 2, 1)
    ).astype(bf)
    # transposed per-core gathered globals: [8, 64, RPC] bf16
    pg = G[pidx % BATCH]                              # [ROWS, 64]
    pgtv = np.ascontiguousarray(
        pg.reshape(N_CORES, RPC, N_GLOBAL).transpose(0, 2, 1)
    ).astype(bf)

    w1xh = W1[:N_FEAT, :].astype(bf)
    w1gh = W1[N_FEAT:, :].astype(bf)
    w2h = W2.astype(bf)
    b1c = np.ascontiguousarray(b1.reshape(2, 128).T)  # [128, 2]
    b1_zero = not np.any(b1)

    in_maps = []
    for c in range(N_CORES):
        in_maps.append({
            "xt": xtv[c],
            "pgt": pgtv[c],
            "w1x": w1xh,
            "w1g": w1gh,
            "w2p": w2h,
            "b1c": b1c,
        })
    return xg, in_maps, b1_zero


def _host_post(xg, results, b2):
    """Assemble full output: [x ; children], adding the exact f32 residual
    repeat(x, 2, axis=-1) and b2 on host."""
    b2 = np.asarray(b2, dtype=np.float32)
    out = np.empty((ROWS + ROWS * N_BR, N_FEAT), dtype=np.float32)
    out[:ROWS] = xg
    # child (global): core c, local parent p, branch br, batch b:
    #   row ROWS + c*CPC + (2p+br)*B + b
    chv = out[ROWS:].reshape(N_CORES, PPC, N_BR, BATCH, N_FEAT)
    xr = xg.reshape(N_CORES, PPC, BATCH, N_FEAT)
    rep_idx = np.arange(N_FEAT) // 2      # residual: channel br*128+f <- x[64*br + f//2]
    for c in range(N_CORES):
        chc = results[c]["ch"].reshape(N_FEAT, N_BR, RPC)
        for br in range(N_BR):
            ffn = chc[:, br, :].astype(np.float32)           # [128, RPC]
            ffn = np.ascontiguousarray(ffn.T).reshape(PPC, BATCH, N_FEAT)
            res = xr[c][:, :, 64 * br + rep_idx]
            chv[c, :, br] = ffn + res + b2[br * N_FEAT:(br + 1) * N_FEAT]
    return out


def kernel(x, global_features, W1, b1, W2, b2, idxs_level, parents_idxs,
           _trace=False, _trace_kwargs=None):
    from concourse.bass_utils import run_bass_kernel_spmd

    xg, in_maps, b1_zero = _host_prep(
        x, global_features, W1, b1, W2, b2, idxs_level, parents_idxs
    )
    nc = _build_program(b1_zero=b1_zero)
    res = run_bass_kernel_spmd(
        nc, in_maps, list(range(N_CORES)),
        trace=_trace, **(_trace_kwargs or {}),
    )
    out = _host_post(xg, res.results, b2)
    if _trace:
        kernel.last_result = res
    return out


# revision 14
# speedup vs baseline: 1.3468x; 1.0310x over previous
"""Trainium2 Bass kernel for nn_BranchingLayer (gnn_message_passing).

Computation (reference):
    parents_ftxs = x[idxs_level]                      # identity gather (arange)
    pg           = global_features[parents_idxs % B]  # random gather
    h1 = leaky_relu([parents_ftxs, pg] @ W1 + b1)
    h2 = h1 @ W2 + b2 + repeat(parents_ftxs, 2, -1)
    children = interleave-reshape(h2)                 # child (2p+br)*B+b, f <- h2[p*B+b, br*F+f]
    out = concat([x, children])

Device strategy (8 cores, 32768 rows/core), fully transposed bf16 dataflow:
  - host: per core, x^T -> [128, RPC] bf16 and gathered global^T -> [64, RPC]
    bf16 (natural column order, column j == local row j).
  - compute in 2-group chunks (group = 512 rows): mm1 (8 MMs, N=512,
    stationary-major so consecutive MMs share lhsT) -> h1^T [128,1024] PSUM
    per group; fused leaky-relu on ACT -> bf16 SBUF; mm2 with W2 chunk
    stationaries -> children^T per branch [128,1024] PSUM per group; fused
    DVE cast -> [128, 2048] bf16 SBUF per chunk; single out-DMA per chunk.
    Input DMAs at 4-group granularity, prefetched one chunk-pair ahead.
    PE issue order is mm1(c) then mm2(c-1) so ACT/DVE overlap the PE.
  - host: un-transpose children, add exact f32 residual repeat(x,2) + b2,
    concat [x, children].
"""

import sys

import numpy as np

try:
    import ml_dtypes
except ImportError:
    ml_dtypes = None

if "/opt/trn_rl_repo" not in sys.path:
    sys.path.insert(0, "/opt/trn_rl_repo")

N_PARENTS = 256
BATCH = 1024
N_FEAT = 128
N_BR = 2
N_GLOBAL = 64
N_CORES = 8
ROWS = N_PARENTS * BATCH            # 262144
RPC = ROWS // N_CORES               # 32768 rows per core
PPC = N_PARENTS // N_CORES          # 32 parents per core
CPC = RPC * N_BR                    # 65536 child rows per core
GROUP = 512                         # rows per compute group
N_GROUPS = RPC // GROUP             # 64
N_CHUNKS = N_GROUPS // 2            # 32 compute chunks (2 groups each)
N_PAIRS = N_CHUNKS // 2             # 16 input-DMA pairs (4 groups each)
HID = 256

_CACHE = {}


def _split_multiwait(nc, mybir):
    """This image's walrus accepts only one sync-wait per instruction; hoist
    extra waits onto same-engine NOPs inserted before the instruction."""
    for f in nc.m.functions:
        for bb in f.blocks:
            new_insts = []
            changed = False
            for inst in bb.instructions:
                si = inst.sync_info
                if si is not None and len(si.on_wait) > 1:
                    waits = list(si.on_wait)
                    for w in waits[:-1]:
                        new_insts.append(
                            mybir.InstNoOp(
                                name=nc.get_next_instruction_name(),
                                engine=inst.engine,
                                sync_info=mybir.SyncInfo(on_wait=[w], on_update=[]),
                            )
                        )
                    inst.sync_info = mybir.SyncInfo(
                        on_wait=[waits[-1]], on_update=list(si.on_update)
                    )
                    changed = True
                new_insts.append(inst)
            if changed:
                bb.instructions = new_insts


def _mark_ldweights_reuse(nc, mybir):
    """Post-schedule pass: the tile framework emits an InstLdweights before
    every matmul.  When consecutive loads target the identical stationary AP
    (only matmuls/NOPs between), the reload is redundant — drop it (or turn
    it into a NoOp if it carries sync waits/updates)."""
    def wkey(inst):
        w = inst.ins[0]
        return (w.memref, w.offset, tuple(tuple(p) for p in w.ap), str(w.dtype),
                bool(inst.is_transpose), str(getattr(inst, "perf_mode", None)))

    removed = 0
    for f in nc.m.functions:
        for bb in f.blocks:
            new_insts = []
            changed = False
            last = None
            for inst in bb.instructions:
                if inst.engine != mybir.EngineType.PE:
                    new_insts.append(inst)
                    continue
                if isinstance(inst, mybir.InstLdweights):
                    k = wkey(inst)
                    if last is not None and k == last:
                        removed += 1
                        changed = True
                        si = inst.sync_info
                        if si is not None and (si.on_wait or si.on_update):
                            new_insts.append(
                                mybir.InstNoOp(
                                    name=nc.get_next_instruction_name(),
                                    engine=inst.engine,
                                    sync_info=si,
                                )
                            )
                        continue
                    last = k
                elif isinstance(inst, (mybir.InstMatmult, mybir.InstNoOp)):
                    pass
                else:
                    last = None
                new_insts.append(inst)
            if changed:
                bb.instructions = new_insts
    return removed


def _build_program(b1_zero=True, split_waits=True):
    key = ("prog_v4", b1_zero, split_waits)
    if key in _CACHE:
        return _CACHE[key]

    import concourse.bass as bass
    import concourse.mybir as mybir
    import concourse.tile as tile

    f32 = mybir.dt.float32
    bf16 = mybir.dt.bfloat16
    AF = mybir.ActivationFunctionType

    nc = bass.Bass()
    xt = nc.declare_dram_parameter("xt", [N_FEAT, RPC], bf16, isOutput=False)
    pgt = nc.declare_dram_parameter("pgt", [N_GLOBAL, RPC], bf16, isOutput=False)
    w1x = nc.declare_dram_parameter("w1x", [N_FEAT, HID], bf16, isOutput=False)
    w1g = nc.declare_dram_parameter("w1g", [N_GLOBAL, HID], bf16, isOutput=False)
    w2p = nc.declare_dram_parameter("w2p", [HID, HID], bf16, isOutput=False)
    b1c = nc.declare_dram_parameter("b1c", [128, 2], f32, isOutput=False)
    ch = nc.declare_dram_parameter("ch", [N_FEAT, N_BR * RPC], bf16, isOutput=True)
    ch_v = ch.rearrange("p (b r) -> p b r", b=N_BR)

    with tile.TileContext(nc) as tc:
        with (
            tc.tile_pool(name="const", bufs=1) as cpool,
            tc.tile_pool(name="xin", bufs=2) as xpool,
            tc.tile_pool(name="pg", bufs=2) as gpool,
            tc.tile_pool(name="h1", bufs=3) as h1pool,
            tc.tile_pool(name="cout", bufs=3) as opool,
            tc.tile_pool(name="psB", bufs=2, space="PSUM") as psB,
            tc.tile_pool(name="psC", bufs=2, space="PSUM") as psC,
        ):
            # weights load on the ACT hwdge ring, in parallel with the first
            # input DMAs on the sync ring
            w1xs = cpool.tile([N_FEAT, HID], bf16)
            nc.scalar.dma_start(w1xs[:], w1x[:, :])
            w1gs = cpool.tile([N_GLOBAL, HID], bf16)
            nc.scalar.dma_start(w1gs[:], w1g[:, :])
            w2s = [cpool.tile([128, HID], bf16, name=f"w2s{k}") for k in range(2)]
            for k in range(2):
                nc.scalar.dma_start(w2s[k][:], w2p[k * 128:(k + 1) * 128, :])
            b1s = cpool.tile([128, 2], f32)
            nc.scalar.dma_start(b1s[:], b1c[:])

            PW = 4 * GROUP          # columns per input pair (4 groups)
            CW = 2 * GROUP          # columns per compute chunk (2 groups)

            def issue_in(p, split=False):
                xtc = xpool.tile([N_FEAT, PW], bf16, tag="xtc", name=f"xtc{p % 2}")
                pgc = gpool.tile([N_GLOBAL, PW], bf16, tag="pgc", name=f"pgc{p % 2}")
                base = p * PW
                if split:  # first pair: land chunk 0's columns first
                    nc.sync.dma_start(xtc[:, 0:CW], xt[:, base:base + CW])
                    nc.sync.dma_start(pgc[:, 0:CW], pgt[:, base:base + CW])
                    nc.sync.dma_start(xtc[:, CW:PW], xt[:, base + CW:base + PW])
                    nc.sync.dma_start(pgc[:, CW:PW], pgt[:, base + CW:base + PW])
                else:
                    nc.sync.dma_start(xtc[:, :], xt[:, base:base + PW])
                    nc.sync.dma_start(pgc[:, :], pgt[:, base:base + PW])
                return xtc, pgc

            pair_tiles = {0: issue_in(0, split=True)}

            prev = None
            for c in range(N_CHUNKS + 1):
                cur = None
                if c < N_CHUNKS:
                    if c % 2 == 0 and (c // 2 + 1) < N_PAIRS:
                        pair_tiles[c // 2 + 1] = issue_in(c // 2 + 1)
                    xtc, pgc = pair_tiles[c // 2]
                    off = (c % 2) * CW
                    xg = [xtc[:, off + g * GROUP: off + (g + 1) * GROUP]
                          for g in range(2)]
                    pgg = [pgc[:, off + g * GROUP: off + (g + 1) * GROUP]
                           for g in range(2)]

                    # ---- mm1(c): h1ps[m] spans both groups (cols g*512+j) so
                    # the scheduler's early-close order pairs stationaries ----
                    h1ps = [psB.tile([128, CW], f32, tag="h1ps", name=f"h1ps{m_}")
                            for m_ in range(2)]
                    cur = {"c": c, "h1ps": h1ps}
                    for m in range(2):
                        for g in range(2):
                            nc.tensor.matmul(
                                h1ps[m][:, g * GROUP:(g + 1) * GROUP],
                                w1xs[:, m * 128:(m + 1) * 128], xg[g],
                                start=True, stop=False,
                            )
                        for g in range(2):
                            nc.tensor.matmul(
                                h1ps[m][:, g * GROUP:(g + 1) * GROUP],
                                w1gs[:, m * 128:(m + 1) * 128], pgg[g],
                                start=False, stop=True,
                            )

                h2ps = None
                if prev is not None:
                    # ---- mm2(c-1): h2ps[br] spans both groups (cols g*512+j) ----
                    h2ps = [psC.tile([128, CW], f32, tag="h2ps", name=f"h2ps{br_}")
                            for br_ in range(2)]
                    h1sb = prev["h1sb"]
                    for br in range(2):
                        for k in range(2):
                            for g in range(2):
                                nc.tensor.matmul(
                                    h2ps[br][:, g * GROUP:(g + 1) * GROUP],
                                    w2s[k][:, br * 128:(br + 1) * 128],
                                    h1sb[k][:, g * GROUP:(g + 1) * GROUP],
                                    start=(k == 0), stop=(k == 1),
                                )

                if cur is not None:
                    # ---- ACT: fused leaky-relu(c) -> bf16, overlaps mm2(c-1).
                    # h1ps[m] has a single hid-half on partitions, so the b1
                    # bias is per-partition and exact for any b1. ----
                    h1sb = [h1pool.tile([128, CW], bf16, tag="h1sb",
                                        name=f"h1sb{m_}") for m_ in range(2)]
                    for m in range(2):
                        nc.scalar.activation(
                            h1sb[m][:, :], cur["h1ps"][m][:, :], AF.Lrelu,
                            bias=b1s[:, m:m + 1], scale=1.0, alpha=0.01,
                        )
                    cur["h1sb"] = h1sb

                if prev is not None:
                    # ---- DVE cast (c-1), contiguous per branch + out-DMA ----
                    pc = prev["c"]
                    cho = opool.tile([128, N_BR * CW], bf16, tag="cho")
                    for br in range(2):
                        nc.vector.tensor_copy(
                            cho[:, br * CW:(br + 1) * CW], h2ps[br][:, :],
                        )
                    if pc == N_CHUNKS - 1:
                        # tail: per-branch DMAs so br0 streams out while br1 casts
                        for br in range(2):
                            nc.sync.dma_start(
                                ch_v[:, br, pc * CW:(pc + 1) * CW],
                                cho[:, br * CW:(br + 1) * CW],
                            )
                    else:
                        nc.sync.dma_start(
                            ch_v[:, :, pc * CW:(pc + 1) * CW],
                            cho[:, :].rearrange("p (b t) -> p b t", b=2),
                        )

                prev = cur

    if split_waits:
        _split_multiwait(nc, mybir)
    _mark_ldweights_reuse(nc, mybir)
    _CACHE[key] = nc
    return nc


def _host_prep(x, global_features, W1, b1, W2, b2, idxs_level, parents_idxs):
    bf = ml_dtypes.bfloat16
    x = np.ascontiguousarray(np.asarray(x, dtype=np.float32))
    G = np.asarray(global_features, dtype=np.float32)
    W1 = np.asarray(W1, dtype=np.float32)
    b1 = np.asarray(b1, dtype=np.float32)
    W2 = np.asarray(W2, dtype=np.float32)
    idxs = np.asarray(idxs_level)
    pidx = np.asarray(parents_idxs)

    if np.array_equal(idxs, np.arange(ROWS, dtype=idxs.dtype)):
        xg = x
    else:  # general gather fallback (host)
        xg = np.ascontiguousarray(x[idxs])

    # transposed per-core x: [8, 128, RPC] bf16, column j == local row j
    xtv = np.ascontiguousarray(
        xg.reshape(N_CORES, RPC, N_FEAT).transpose(0, 2, 1)
    ).astype(bf)
    # transposed per-core gathered globals: [8, 64, RPC] bf16
    pg = G[pidx % BATCH]                              # [ROWS, 64]
    pgtv = np.ascontiguousarray(
        pg.reshape(N_CORES, RPC, N_GLOBAL).transpose(0, 2, 1)
    ).astype(bf)

    w1xh = W1[:N_FEAT, :].astype(bf)
    w1gh = W1[N_FEAT:, :].astype(bf)
    w2h = W2.astype(bf)
    b1c = np.ascontiguousarray(b1.reshape(2, 128).T)  # [128, 2]
    b1_zero = not np.any(b1)

    in_maps = []
    for c in range(N_CORES):
        in_maps.append({
            "xt": xtv[c],
            "pgt": pgtv[c],
            "w1x": w1xh,
            "w1g": w1gh,
            "w2p": w2h,
            "b1c": b1c,
        })
    return xg, in_maps, b1_zero


def _host_post(xg, results, b2):
    """Assemble full output: [x ; children], adding the exact f32 residual
    repeat(x, 2, axis=-1) and b2 on host."""
    b2 = np.asarray(b2, dtype=np.float32)
    out = np.empty((ROWS + ROWS * N_BR, N_FEAT), dtype=np.float32)
    out[:ROWS] = xg
    # child (global): core c, local parent p, branch br, batch b:
    #   row ROWS + c*CPC + (2p+br)*B + b
    chv = out[ROWS:].reshape(N_CORES, PPC, N_BR, BATCH, N_FEAT)
    xr = xg.reshape(N_CORES, PPC, BATCH, N_FEAT)
    rep_idx = np.arange(N_FEAT) // 2      # residual: channel br*128+f <- x[64*br + f//2]
    for c in range(N_CORES):
        chc = results[c]["ch"].reshape(N_FEAT, N_BR, RPC)
        for br in range(N_BR):
            ffn = chc[:, br, :].astype(np.float32)           # [128, RPC]
            ffn = np.ascontiguousarray(ffn.T).reshape(PPC, BATCH, N_FEAT)
            res = xr[c][:, :, 64 * br + rep_idx]
            chv[c, :, br] = ffn + res + b2[br * N_FEAT:(br + 1) * N_FEAT]
    return out


def kernel(x, global_features, W1, b1, W2, b2, idxs_level, parents_idxs,
           _trace=False, _trace_kwargs=None):
    from concourse.bass_utils import run_bass_kernel_spmd

    xg, in_maps, b1_zero = _host_prep(
        x, global_features, W1, b1, W2, b2, idxs_level, parents_idxs
    )
    nc = _build_program(b1_zero=b1_zero)
    res = run_bass_kernel_spmd(
        nc, in_maps, list(range(N_CORES)),
        trace=_trace, **(_trace_kwargs or {}),
    )
    out = _host_post(xg, res.results, b2)
    if _trace:
        kernel.last_result = res
    return out
